# revision 1
# baseline (speedup 1.0000x reference)
"""Chamfer distance loss on 8 Trainium2 NeuronCores.

Strategy (hardcoded for point clouds [1, 16384, 128] f32):
  - Shard point_cloud1 rows across 8 cores (2048 rows each); replicate
    point_cloud2.
  - Per core, PE computes psum tiles of the full squared-distance matrix
    dist[i,j] = a2[i] + b2[j] - 2*a.b via one f32r matmul (K=128) plus one
    fp16 rank-2 matmul (a2/ones | ones/b2, zero-padded to K=128 — K<128
    accumulate groups corrupt 16-bit PSUM reads on this silicon).
  - ScalarE evacuates PSUM -> fp16 SBUF tiles (bias -256 recenters for fp16
    precision).
  - VectorE: direction-1 row mins via 2x-mode fp16 min-trees (pair-min per
    group pair, then an in-place binary tree); direction-2 column mins via
    2x-mode fp16 elementwise-min accumulation.
  - Host: mean of row mins + mean over cores/partitions of column mins,
    +256 recenter (a2/b2 already folded into the distance tiles).
"""
import numpy as np

N = 16384
D = 128
P = 128
NCORES = 8
MPC = N // NCORES          # rows per core = 2048
MCH = MPC // P             # row chunks per core = 16
NGRP = 8                   # column groups
GW = N // NGRP             # group width = 2048
NPAIR = NGRP // 2          # group pairs per stripe = 4
CENTER = 256.0

_CACHE = {}


def _build(repeat=1):
    from contextlib import ExitStack
    import concourse.bacc as bacc
    import concourse.tile as tile
    from concourse import mybir

    f32 = mybir.dt.float32
    f16 = mybir.dt.float16
    f32r = mybir.dt.float32r
    MIN = mybir.AluOpType.min

    nc = bacc.Bacc(trn_type="TRN2", target_bir_lowering=False, debug=False,
                   num_devices=NCORES)

    at_d = nc.dram_tensor("at", [D, MPC], f32r, kind="ExternalInput").ap()
    bt_d = nc.dram_tensor("bt", [D, N], f32r, kind="ExternalInput").ap()
    a2p_d = nc.dram_tensor("a2p", [D, MPC], f16, kind="ExternalInput").ap()
    ob2p_d = nc.dram_tensor("ob2p", [D, N], f16, kind="ExternalInput").ap()
    rm_d = nc.dram_tensor("rm", [P, MCH], f32, kind="ExternalOutput").ap()
    cm_d = nc.dram_tensor("cm", [P, N], f16, kind="ExternalOutput").ap()

    with tile.TileContext(nc) as tc, ExitStack() as ctx:
        cpool = ctx.enter_context(tc.tile_pool(name="const", bufs=1))
        psum_pool = ctx.enter_context(tc.tile_pool(name="psum", bufs=2, space="PSUM"))
        tpool = ctx.enter_context(tc.tile_pool(name="tg", bufs=2))

        BT = cpool.tile([D, N], f32r)
        AT = cpool.tile([D, MPC], f32r)
        A2P = cpool.tile([D, MPC], f16)
        OB2P = cpool.tile([D, N], f16)
        ACC = cpool.tile([P, N], f16)
        RM = cpool.tile([P, MCH], f32)
        S = cpool.tile([P, N // 2], f16)

        nc.sync.dma_start(AT[:], at_d[:])
        nc.sync.dma_start(A2P[:], a2p_d[:])
        for g in range(NGRP):
            sl = slice(g * GW, (g + 1) * GW)
            nc.sync.dma_start(BT[:, sl], bt_d[:, sl])
            nc.sync.dma_start(OB2P[:, sl], ob2p_d[:, sl])
        from contextlib import nullcontext
        loop_ctx = tc.For_i(0, repeat, 1) if repeat > 1 else nullcontext()
        with loop_ctx:
            nc.vector.memset(ACC[:], 60000.0)
            for m in range(MCH):
                msl = slice(m * P, (m + 1) * P)
                for h in range(NPAIR):
                    T = tpool.tile([P, 2 * GW], f16)
                    for half in range(2):
                        g = 2 * h + half
                        ps = psum_pool.tile([P, GW], f32)
                        for k in range(4):
                            nsl = slice(g * GW + k * 512, g * GW + (k + 1) * 512)
                            ksl = slice(k * 512, (k + 1) * 512)
                            nc.tensor.matmul(ps[:, ksl], AT[:, msl], BT[:, nsl],
                                             start=True, stop=False)
                        for k in range(4):
                            nsl = slice(g * GW + k * 512, g * GW + (k + 1) * 512)
                            ksl = slice(k * 512, (k + 1) * 512)
                            nc.tensor.matmul(ps[:, ksl], A2P[:, msl], OB2P[:, nsl],
                                             start=False, stop=True)
                        nc.scalar.activation(T[:, half * GW:(half + 1) * GW], ps[:],
                                             mybir.ActivationFunctionType.Copy,
                                             bias=-CENTER)
                    # direction 2: column-min accumulate over row chunks
                    gsl2 = slice(2 * h * GW, (2 * h + 2) * GW)
                    nc.vector.tensor_tensor(out=ACC[:, gsl2], in0=ACC[:, gsl2],
                                            in1=T[:], op=MIN)
                    # direction 1, level 1: pair-min into the stripe buffer
                    hsl = slice(h * GW, (h + 1) * GW)
                    nc.vector.tensor_tensor(out=S[:, hsl], in0=T[:, :GW],
                                            in1=T[:, GW:], op=MIN)
                # direction 1: in-place min-tree over S, then final reduce
                w = N // 4
                while w >= 16:
                    nc.vector.tensor_tensor(out=S[:, :w], in0=S[:, :w],
                                            in1=S[:, w:2 * w], op=MIN)
                    w //= 2
                nc.vector.tensor_reduce(out=RM[:, m:m + 1], in_=S[:, :16],
                                        axis=mybir.AxisListType.X, op=MIN)

        nc.sync.dma_start(rm_d[:], RM[:])
        for g in range(NGRP):
            sl = slice(g * GW, (g + 1) * GW)
            nc.sync.dma_start(cm_d[:, sl], ACC[:, sl])

    nc.compile()
    return nc


def kernel(point_cloud1: np.ndarray, point_cloud2: np.ndarray) -> np.ndarray:
    import os
    from concourse.bass_utils import run_bass_kernel_spmd

    if "nc" not in _CACHE:
        _CACHE["nc"] = _build()
    nc = _CACHE["nc"]

    pc1 = np.ascontiguousarray(np.asarray(point_cloud1).reshape(N, D),
                               dtype=np.float32)
    pc2 = np.ascontiguousarray(np.asarray(point_cloud2).reshape(N, D),
                               dtype=np.float32)
    a2 = (pc1.astype(np.float64) ** 2).sum(1).astype(np.float32)
    b2 = (pc2.astype(np.float64) ** 2).sum(1).astype(np.float32)

    bt = np.ascontiguousarray(pc2.T)
    ob2p = np.zeros((D, N), np.float16)
    ob2p[0] = 1.0
    ob2p[1] = b2.astype(np.float16)

    in_maps = []
    for c in range(NCORES):
        rs = slice(c * MPC, (c + 1) * MPC)
        a2p = np.zeros((D, MPC), np.float16)
        a2p[0] = a2[rs].astype(np.float16)
        a2p[1] = 1.0
        in_maps.append({
            "at": np.ascontiguousarray(-2.0 * pc1[rs].T),
            "bt": bt,
            "a2p": a2p,
            "ob2p": ob2p,
        })

    trace = os.environ.get("KERNEL_TRACE", "0") == "1"
    if trace:
        try:
            import antenv.axon_hooks  # noqa: F401
        except ImportError:
            trace = False
    res = run_bass_kernel_spmd(nc, in_maps, core_ids=list(range(NCORES)),
                               trace=trace)
    _CACHE["last_exec_ns"] = res.exec_time_ns

    rowmins = []
    colmins = []
    for r in res.results:
        # rm[p, m] is the row-min of core row m*128+p, minus CENTER
        rowmins.append(r["rm"].T.reshape(MPC))
        colmins.append(r["cm"].astype(np.float32))
    min1 = np.concatenate(rowmins) + CENTER
    min2 = np.concatenate(colmins, axis=0).min(axis=0) + CENTER
    out = np.float64(min1.mean()) + np.float64(min2.mean())
    return np.asarray(out, dtype=np.float32)



# revision 5
# speedup vs baseline: 1.1953x; 1.1953x over previous
"""Chamfer distance loss on 8 Trainium2 NeuronCores.

Strategy (hardcoded for point clouds [1, 16384, 128] f32):
  - Shard point_cloud1 rows across 8 cores (2048 rows each); replicate
    point_cloud2.
  - Per core, PE computes psum tiles of the full squared-distance matrix
    dist[i,j] = a2[i] + b2[j] - 2*a.b via one bf16 matmul (K=128) plus one
    fp16 rank-2 matmul (a2/ones | ones/b2, zero-padded to K=128 — K<128
    accumulate groups corrupt 16-bit PSUM reads on this silicon).
  - ScalarE evacuates PSUM -> fp16 SBUF tiles (bias -256 recenters for fp16
    precision).
  - VectorE: direction-2 column mins via 2x-mode fp16 elementwise-min
    accumulation; direction-1 row mins via tensor_tensor_reduce (pair-min of
    stripe halves fused with a chained free-dim min-reduce into RM).
  - Host: mean of row mins + mean over cores/partitions of column mins,
    +256 recenter (a2/b2 already folded into the distance tiles).
"""
import numpy as np

N = 16384
D = 128
P = 128
NCORES = 8
MPC = N // NCORES          # rows per core = 2048
MCH = MPC // P             # row chunks per core = 16
NGRP = 8                   # column groups
GW = N // NGRP             # group width = 2048
NPAIR = NGRP // 2          # group pairs per stripe = 4
CENTER = 256.0

_CACHE = {}


def _build(repeat=1):
    from contextlib import ExitStack
    import concourse.bacc as bacc
    import concourse.tile as tile
    from concourse import mybir

    f32 = mybir.dt.float32
    f16 = mybir.dt.float16
    MIN = mybir.AluOpType.min

    nc = bacc.Bacc(trn_type="TRN2", target_bir_lowering=False, debug=False,
                   num_devices=NCORES)

    at_d = nc.dram_tensor("at", [D, MPC], f16, kind="ExternalInput").ap()
    bt_d = nc.dram_tensor("bt", [D, N], f16, kind="ExternalInput").ap()
    a2p_d = nc.dram_tensor("a2p", [D, MPC], f16, kind="ExternalInput").ap()
    ob2p_d = nc.dram_tensor("ob2p", [D, N], f16, kind="ExternalInput").ap()
    rm_d = nc.dram_tensor("rm", [P, MCH], f32, kind="ExternalOutput").ap()
    cm_d = nc.dram_tensor("cm", [P, N], f16, kind="ExternalOutput").ap()

    with tile.TileContext(nc) as tc, ExitStack() as ctx:
        cpool = ctx.enter_context(tc.tile_pool(name="const", bufs=1))
        psum_pool = ctx.enter_context(tc.tile_pool(name="psum", bufs=2, space="PSUM"))
        tpool = ctx.enter_context(tc.tile_pool(name="tg", bufs=2))

        BT = cpool.tile([D, N], f16)
        AT = cpool.tile([D, MPC], f16)
        A2P = cpool.tile([D, MPC], f16)
        OB2P = cpool.tile([D, N], f16)
        ACC = cpool.tile([P, N], f16)
        RM = cpool.tile([P, MCH], f32)
        S = cpool.tile([P, N // 2], f16)

        nc.sync.dma_start(AT[:], at_d[:])
        nc.sync.dma_start(A2P[:], a2p_d[:])
        for g in range(NGRP):
            sl = slice(g * GW, (g + 1) * GW)
            nc.sync.dma_start(BT[:, sl], bt_d[:, sl])
            nc.sync.dma_start(OB2P[:, sl], ob2p_d[:, sl])
        from contextlib import nullcontext
        loop_ctx = tc.For_i(0, repeat, 1) if repeat > 1 else nullcontext()
        with loop_ctx:
            nc.vector.memset(ACC[:], 60000.0)
            for m in range(MCH):
                msl = slice(m * P, (m + 1) * P)
                for h in range(NPAIR):
                    T = tpool.tile([P, 2 * GW], f16)
                    for half in range(2):
                        g = 2 * h + half
                        ps = psum_pool.tile([P, GW], f32)
                        for k in range(4):
                            nsl = slice(g * GW + k * 512, g * GW + (k + 1) * 512)
                            ksl = slice(k * 512, (k + 1) * 512)
                            nc.tensor.matmul(ps[:, ksl], AT[:, msl], BT[:, nsl],
                                             start=True, stop=False)
                        for k in range(4):
                            nsl = slice(g * GW + k * 512, g * GW + (k + 1) * 512)
                            ksl = slice(k * 512, (k + 1) * 512)
                            nc.tensor.matmul(ps[:, ksl], A2P[:, msl], OB2P[:, nsl],
                                             start=False, stop=True)
                        nc.scalar.activation(T[:, half * GW:(half + 1) * GW], ps[:],
                                             mybir.ActivationFunctionType.Copy,
                                             bias=-CENTER)
                    # direction 2: column-min accumulate over row chunks
                    gsl2 = slice(2 * h * GW, (2 * h + 2) * GW)
                    nc.vector.tensor_tensor(out=ACC[:, gsl2], in0=ACC[:, gsl2],
                                            in1=T[:], op=MIN)
                    # direction 1, level 1: pair-min into the stripe buffer
                    hsl = slice(h * GW, (h + 1) * GW)
                    nc.vector.tensor_tensor(out=S[:, hsl], in0=T[:, :GW],
                                            in1=T[:, GW:], op=MIN)
                # direction 1: in-place min-tree over S, then final reduce
                w = N // 4
                while w >= 16:
                    nc.vector.tensor_tensor(out=S[:, :w], in0=S[:, :w],
                                            in1=S[:, w:2 * w], op=MIN)
                    w //= 2
                nc.vector.tensor_reduce(out=RM[:, m:m + 1], in_=S[:, :16],
                                        axis=mybir.AxisListType.X, op=MIN)

        nc.sync.dma_start(rm_d[:], RM[:])
        for g in range(NGRP):
            sl = slice(g * GW, (g + 1) * GW)
            nc.sync.dma_start(cm_d[:, sl], ACC[:, sl])

    nc.compile()
    return nc


def kernel(point_cloud1: np.ndarray, point_cloud2: np.ndarray) -> np.ndarray:
    import os
    from concourse.bass_utils import run_bass_kernel_spmd

    if "nc" not in _CACHE:
        _CACHE["nc"] = _build()
    nc = _CACHE["nc"]

    pc1 = np.ascontiguousarray(np.asarray(point_cloud1).reshape(N, D),
                               dtype=np.float32)
    pc2 = np.ascontiguousarray(np.asarray(point_cloud2).reshape(N, D),
                               dtype=np.float32)
    a2 = (pc1.astype(np.float64) ** 2).sum(1).astype(np.float32)
    b2 = (pc2.astype(np.float64) ** 2).sum(1).astype(np.float32)

    bt = np.ascontiguousarray(pc2.T).astype(np.float16)
    ob2p = np.zeros((D, N), np.float16)
    ob2p[0] = 1.0
    ob2p[1] = b2.astype(np.float16)

    in_maps = []
    for c in range(NCORES):
        rs = slice(c * MPC, (c + 1) * MPC)
        a2p = np.zeros((D, MPC), np.float16)
        a2p[0] = a2[rs].astype(np.float16)
        a2p[1] = 1.0
        in_maps.append({
            "at": np.ascontiguousarray(-2.0 * pc1[rs].T).astype(np.float16),
            "bt": bt,
            "a2p": a2p,
            "ob2p": ob2p,
        })

    trace = os.environ.get("KERNEL_TRACE", "0") == "1"
    if trace:
        try:
            import antenv.axon_hooks  # noqa: F401
        except ImportError:
            trace = False
    res = run_bass_kernel_spmd(nc, in_maps, core_ids=list(range(NCORES)),
                               trace=trace)
    _CACHE["last_exec_ns"] = res.exec_time_ns

    rowmins = []
    colmins = []
    for r in res.results:
        # rm[p, m] is the row-min of core row m*128+p, minus CENTER
        rowmins.append(r["rm"].T.reshape(MPC))
        colmins.append(r["cm"].astype(np.float32))
    min1 = np.concatenate(rowmins) + CENTER
    min2 = np.concatenate(colmins, axis=0).min(axis=0) + CENTER
    out = np.float64(min1.mean()) + np.float64(min2.mean())
    return np.asarray(out, dtype=np.float32)


# revision 9
# speedup vs baseline: 1.3348x; 1.1167x over previous
"""Chamfer distance loss on 8 Trainium2 NeuronCores.

Strategy (hardcoded for point clouds [1, 16384, 128] f32):
  - Shard point_cloud1 rows across 8 cores (2048 rows each); replicate
    point_cloud2.
  - Per core, PE computes psum tiles of the full squared-distance matrix
    dist[i,j] = a2[i] + b2[j] - 2*a.b via one fp16 matmul (K=128) plus one
    fp16 rank-2 matmul (a2/ones | ones/b2, zero-padded to K=128 — K<128
    accumulate groups corrupt 16-bit PSUM reads on this silicon).
  - ScalarE evacuates PSUM -> fp16 SBUF tiles (bias -256 recenters for fp16
    precision).
  - VectorE: direction-2 column mins via 2x-mode fp16 elementwise-min
    accumulation; direction-1 row mins via 2x-mode fp16 pair-min of stripe
    halves followed by an in-place binary min-tree.
    NOTE: tensor_tensor_reduce crashes the device at runtime on this silicon
    (3/3 attempts, INTERNAL error) — do not use it here.
  - Host: mean of row mins + mean over cores/partitions of column mins,
    +256 recenter (a2/b2 already folded into the distance tiles).
"""
import numpy as np

N = 16384
D = 128
P = 128
NCORES = 8
MPC = N // NCORES          # rows per core = 2048
MCH = MPC // P             # row chunks per core = 16
NGRP = 8                   # column groups
GW = N // NGRP             # group width = 2048
NPAIR = NGRP // 2          # group pairs per stripe = 4
CENTER = 256.0

_CACHE = {}


def _build(repeat=1):
    from contextlib import ExitStack
    import concourse.bacc as bacc
    import concourse.tile as tile
    from concourse import mybir

    f32 = mybir.dt.float32
    f16 = mybir.dt.float16
    MIN = mybir.AluOpType.min

    nc = bacc.Bacc(trn_type="TRN2", target_bir_lowering=False, debug=False,
                   num_devices=NCORES)

    at_d = nc.dram_tensor("at", [D, MPC], f16, kind="ExternalInput").ap()
    bt_d = nc.dram_tensor("bt", [D, N], f16, kind="ExternalInput").ap()
    a2p_d = nc.dram_tensor("a2p", [D, MPC], f16, kind="ExternalInput").ap()
    ob2p_d = nc.dram_tensor("ob2p", [D, N], f16, kind="ExternalInput").ap()
    rm_d = nc.dram_tensor("rm", [P, MCH], f32, kind="ExternalOutput").ap()
    cm_d = nc.dram_tensor("cm", [P, N], f16, kind="ExternalOutput").ap()

    with tile.TileContext(nc) as tc, ExitStack() as ctx:
        cpool = ctx.enter_context(tc.tile_pool(name="const", bufs=1))
        psum_pool = ctx.enter_context(tc.tile_pool(name="psum", bufs=2, space="PSUM"))
        tpool = ctx.enter_context(tc.tile_pool(name="tg", bufs=4))

        BT = cpool.tile([D, N], f16)
        AT = cpool.tile([D, MPC], f16)
        A2P = cpool.tile([D, MPC], f16)
        OB2P = cpool.tile([D, N], f16)
        ACC = cpool.tile([P, N], f16)
        RM = cpool.tile([P, MCH], f32)
        S = cpool.tile([P, N // 2], f16)

        nc.sync.dma_start(AT[:], at_d[:])
        nc.sync.dma_start(A2P[:], a2p_d[:])
        for g in range(NGRP):
            sl = slice(g * GW, (g + 1) * GW)
            nc.sync.dma_start(BT[:, sl], bt_d[:, sl])
            nc.sync.dma_start(OB2P[:, sl], ob2p_d[:, sl])
        from contextlib import nullcontext
        loop_ctx = tc.For_i(0, repeat, 1) if repeat > 1 else nullcontext()
        with loop_ctx:
            nc.vector.memset(ACC[:], 60000.0)
            for m in range(MCH):
                msl = slice(m * P, (m + 1) * P)
                for h in range(NPAIR):
                    T = tpool.tile([P, 2 * GW], f16)
                    for half in range(2):
                        g = 2 * h + half
                        ps = psum_pool.tile([P, GW], f32)
                        for k in range(4):
                            nsl = slice(g * GW + k * 512, g * GW + (k + 1) * 512)
                            ksl = slice(k * 512, (k + 1) * 512)
                            nc.tensor.matmul(ps[:, ksl], AT[:, msl], BT[:, nsl],
                                             start=True, stop=False)
                        for k in range(4):
                            nsl = slice(g * GW + k * 512, g * GW + (k + 1) * 512)
                            ksl = slice(k * 512, (k + 1) * 512)
                            nc.tensor.matmul(ps[:, ksl], A2P[:, msl], OB2P[:, nsl],
                                             start=False, stop=True)
                        nc.scalar.activation(T[:, half * GW:(half + 1) * GW], ps[:],
                                             mybir.ActivationFunctionType.Copy,
                                             bias=-CENTER)
                    # direction 2: column-min accumulate over row chunks
                    gsl2 = slice(2 * h * GW, (2 * h + 2) * GW)
                    nc.vector.tensor_tensor(out=ACC[:, gsl2], in0=ACC[:, gsl2],
                                            in1=T[:], op=MIN)
                    # direction 1, level 1: pair-min into the stripe buffer
                    hsl = slice(h * GW, (h + 1) * GW)
                    nc.vector.tensor_tensor(out=S[:, hsl], in0=T[:, :GW],
                                            in1=T[:, GW:], op=MIN)
                # direction 1: in-place min-tree over S, then final reduce
                w = N // 4
                while w >= 16:
                    nc.vector.tensor_tensor(out=S[:, :w], in0=S[:, :w],
                                            in1=S[:, w:2 * w], op=MIN)
                    w //= 2
                nc.vector.tensor_reduce(out=RM[:, m:m + 1], in_=S[:, :16],
                                        axis=mybir.AxisListType.X, op=MIN)

        nc.sync.dma_start(rm_d[:], RM[:])
        for g in range(NGRP):
            sl = slice(g * GW, (g + 1) * GW)
            nc.sync.dma_start(cm_d[:, sl], ACC[:, sl])

    nc.compile()
    return nc


def kernel(point_cloud1: np.ndarray, point_cloud2: np.ndarray) -> np.ndarray:
    import os
    from concourse.bass_utils import run_bass_kernel_spmd

    if "nc" not in _CACHE:
        _CACHE["nc"] = _build()
    nc = _CACHE["nc"]

    pc1 = np.ascontiguousarray(np.asarray(point_cloud1).reshape(N, D),
                               dtype=np.float32)
    pc2 = np.ascontiguousarray(np.asarray(point_cloud2).reshape(N, D),
                               dtype=np.float32)
    a2 = (pc1.astype(np.float64) ** 2).sum(1).astype(np.float32)
    b2 = (pc2.astype(np.float64) ** 2).sum(1).astype(np.float32)

    bt = np.ascontiguousarray(pc2.T).astype(np.float16)
    ob2p = np.zeros((D, N), np.float16)
    ob2p[0] = 1.0
    ob2p[1] = b2.astype(np.float16)

    in_maps = []
    for c in range(NCORES):
        rs = slice(c * MPC, (c + 1) * MPC)
        a2p = np.zeros((D, MPC), np.float16)
        a2p[0] = a2[rs].astype(np.float16)
        a2p[1] = 1.0
        in_maps.append({
            "at": np.ascontiguousarray(-2.0 * pc1[rs].T).astype(np.float16),
            "bt": bt,
            "a2p": a2p,
            "ob2p": ob2p,
        })

    trace = os.environ.get("KERNEL_TRACE", "0") == "1"
    if trace:
        try:
            import antenv.axon_hooks  # noqa: F401
        except ImportError:
            trace = False
    res = run_bass_kernel_spmd(nc, in_maps, core_ids=list(range(NCORES)),
                               trace=trace)
    _CACHE["last_exec_ns"] = res.exec_time_ns

    rowmins = []
    colmins = []
    for r in res.results:
        # rm[p, m] is the row-min of core row m*128+p, minus CENTER
        rowmins.append(r["rm"].T.reshape(MPC))
        colmins.append(r["cm"].astype(np.float32))
    min1 = np.concatenate(rowmins) + CENTER
    min2 = np.concatenate(colmins, axis=0).min(axis=0) + CENTER
    out = np.float64(min1.mean()) + np.float64(min2.mean())
    return np.asarray(out, dtype=np.float32)


# revision 10
# speedup vs baseline: 1.3409x; 1.0046x over previous
"""Chamfer distance loss on 8 Trainium2 NeuronCores.

Strategy (hardcoded for point clouds [1, 16384, 128] f32):
  - Shard point_cloud1 rows across 8 cores (2048 rows each); replicate
    point_cloud2.
  - Per core, PE computes psum tiles of the full squared-distance matrix
    dist[i,j] = a2[i] + b2[j] - 2*a.b via one fp16 matmul (K=128) plus one
    fp16 rank-2 matmul (a2/ones | ones/b2, zero-padded to K=128 — K<128
    accumulate groups corrupt 16-bit PSUM reads on this silicon).
  - ScalarE evacuates PSUM -> fp16 SBUF tiles (bias -256 recenters for fp16
    precision).
  - VectorE: direction-2 column mins via 2x-mode fp16 elementwise-min
    accumulation; direction-1 row mins via 2x-mode fp16 pair-min of stripe
    halves followed by an in-place binary min-tree.
    NOTE: tensor_tensor_reduce crashes the device at runtime on this silicon
    (3/3 attempts, INTERNAL error) — do not use it here.
  - Host: mean of row mins + mean over cores/partitions of column mins,
    +256 recenter (a2/b2 already folded into the distance tiles).
"""
import numpy as np

N = 16384
D = 128
P = 128
NCORES = 8
MPC = N // NCORES          # rows per core = 2048
MCH = MPC // P             # row chunks per core = 16
NGRP = 8                   # column groups
GW = N // NGRP             # group width = 2048
NPAIR = NGRP // 2          # group pairs per stripe = 4
CENTER = 256.0

_CACHE = {}


def _build(repeat=1):
    from contextlib import ExitStack
    import concourse.bacc as bacc
    import concourse.tile as tile
    from concourse import mybir

    f32 = mybir.dt.float32
    f16 = mybir.dt.float16
    MIN = mybir.AluOpType.min

    nc = bacc.Bacc(trn_type="TRN2", target_bir_lowering=False, debug=False,
                   num_devices=NCORES)

    at_d = nc.dram_tensor("at", [D, MPC], f16, kind="ExternalInput").ap()
    bt_d = nc.dram_tensor("bt", [D, N], f16, kind="ExternalInput").ap()
    a2p_d = nc.dram_tensor("a2p", [D, MPC], f16, kind="ExternalInput").ap()
    ob2p_d = nc.dram_tensor("ob2p", [D, N], f16, kind="ExternalInput").ap()
    rm_d = nc.dram_tensor("rm", [P, MCH], f32, kind="ExternalOutput").ap()
    cm_d = nc.dram_tensor("cm", [P, N], f16, kind="ExternalOutput").ap()

    with tile.TileContext(nc) as tc, ExitStack() as ctx:
        cpool = ctx.enter_context(tc.tile_pool(name="const", bufs=1))
        psum_pool = ctx.enter_context(tc.tile_pool(name="psum", bufs=2, space="PSUM"))
        tpool = ctx.enter_context(tc.tile_pool(name="tg", bufs=6))

        BT = cpool.tile([D, N], f16)
        AT = cpool.tile([D, MPC], f16)
        A2P = cpool.tile([D, MPC], f16)
        OB2P = cpool.tile([D, N], f16)
        ACC = cpool.tile([P, N], f16)
        RM = cpool.tile([P, MCH], f32)
        S = cpool.tile([P, N // 2], f16)

        nc.sync.dma_start(AT[:], at_d[:])
        nc.sync.dma_start(A2P[:], a2p_d[:])
        for g in range(NGRP):
            sl = slice(g * GW, (g + 1) * GW)
            nc.sync.dma_start(BT[:, sl], bt_d[:, sl])
            nc.sync.dma_start(OB2P[:, sl], ob2p_d[:, sl])
        from contextlib import nullcontext
        loop_ctx = tc.For_i(0, repeat, 1) if repeat > 1 else nullcontext()
        with loop_ctx:
            nc.vector.memset(ACC[:], 60000.0)
            for m in range(MCH):
                msl = slice(m * P, (m + 1) * P)
                for h in range(NPAIR):
                    T = tpool.tile([P, 2 * GW], f16)
                    for half in range(2):
                        g = 2 * h + half
                        ps = psum_pool.tile([P, GW], f32)
                        for k in range(4):
                            nsl = slice(g * GW + k * 512, g * GW + (k + 1) * 512)
                            ksl = slice(k * 512, (k + 1) * 512)
                            nc.tensor.matmul(ps[:, ksl], AT[:, msl], BT[:, nsl],
                                             start=True, stop=False)
                        for k in range(4):
                            nsl = slice(g * GW + k * 512, g * GW + (k + 1) * 512)
                            ksl = slice(k * 512, (k + 1) * 512)
                            nc.tensor.matmul(ps[:, ksl], A2P[:, msl], OB2P[:, nsl],
                                             start=False, stop=True)
                        nc.scalar.activation(T[:, half * GW:(half + 1) * GW], ps[:],
                                             mybir.ActivationFunctionType.Copy,
                                             bias=-CENTER)
                    # direction 2: column-min accumulate over row chunks
                    gsl2 = slice(2 * h * GW, (2 * h + 2) * GW)
                    nc.vector.tensor_tensor(out=ACC[:, gsl2], in0=ACC[:, gsl2],
                                            in1=T[:], op=MIN)
                    # direction 1, level 1: pair-min into the stripe buffer
                    hsl = slice(h * GW, (h + 1) * GW)
                    nc.vector.tensor_tensor(out=S[:, hsl], in0=T[:, :GW],
                                            in1=T[:, GW:], op=MIN)
                # direction 1: in-place min-tree over S, then final reduce
                w = N // 4
                while w >= 256:
                    nc.vector.tensor_tensor(out=S[:, :w], in0=S[:, :w],
                                            in1=S[:, w:2 * w], op=MIN)
                    w //= 2
                nc.vector.tensor_reduce(out=RM[:, m:m + 1], in_=S[:, :256],
                                        axis=mybir.AxisListType.X, op=MIN)

        nc.sync.dma_start(rm_d[:], RM[:])
        for g in range(NGRP):
            sl = slice(g * GW, (g + 1) * GW)
            nc.sync.dma_start(cm_d[:, sl], ACC[:, sl])

    nc.compile()
    return nc


def kernel(point_cloud1: np.ndarray, point_cloud2: np.ndarray) -> np.ndarray:
    import os
    from concourse.bass_utils import run_bass_kernel_spmd

    if "nc" not in _CACHE:
        _CACHE["nc"] = _build()
    nc = _CACHE["nc"]

    pc1 = np.ascontiguousarray(np.asarray(point_cloud1).reshape(N, D),
                               dtype=np.float32)
    pc2 = np.ascontiguousarray(np.asarray(point_cloud2).reshape(N, D),
                               dtype=np.float32)
    a2 = (pc1.astype(np.float64) ** 2).sum(1).astype(np.float32)
    b2 = (pc2.astype(np.float64) ** 2).sum(1).astype(np.float32)

    bt = np.ascontiguousarray(pc2.T).astype(np.float16)
    ob2p = np.zeros((D, N), np.float16)
    ob2p[0] = 1.0
    ob2p[1] = b2.astype(np.float16)

    in_maps = []
    for c in range(NCORES):
        rs = slice(c * MPC, (c + 1) * MPC)
        a2p = np.zeros((D, MPC), np.float16)
        a2p[0] = a2[rs].astype(np.float16)
        a2p[1] = 1.0
        in_maps.append({
            "at": np.ascontiguousarray(-2.0 * pc1[rs].T).astype(np.float16),
            "bt": bt,
            "a2p": a2p,
            "ob2p": ob2p,
        })

    trace = os.environ.get("KERNEL_TRACE", "0") == "1"
    if trace:
        try:
            import antenv.axon_hooks  # noqa: F401
        except ImportError:
            trace = False
    res = run_bass_kernel_spmd(nc, in_maps, core_ids=list(range(NCORES)),
                               trace=trace)
    _CACHE["last_exec_ns"] = res.exec_time_ns

    rowmins = []
    colmins = []
    for r in res.results:
        # rm[p, m] is the row-min of core row m*128+p, minus CENTER
        rowmins.append(r["rm"].T.reshape(MPC))
        colmins.append(r["cm"].astype(np.float32))
    min1 = np.concatenate(rowmins) + CENTER
    min2 = np.concatenate(colmins, axis=0).min(axis=0) + CENTER
    out = np.float64(min1.mean()) + np.float64(min2.mean())
    return np.asarray(out, dtype=np.float32)


# revision 11
# speedup vs baseline: 1.3918x; 1.0380x over previous
"""Chamfer distance loss on 8 Trainium2 NeuronCores.

Strategy (hardcoded for point clouds [1, 16384, 128] f32):
  - Shard point_cloud1 rows across 8 cores (2048 rows each); replicate
    point_cloud2.
  - Per core, PE computes psum tiles of the full squared-distance matrix
    dist[i,j] = a2[i] + b2[j] - 2*a.b via one fp16 matmul (K=128) plus one
    fp16 rank-2 matmul (a2/ones | ones/b2, zero-padded to K=128 — K<128
    accumulate groups corrupt 16-bit PSUM reads on this silicon).
  - ScalarE evacuates PSUM -> fp16 SBUF tiles (bias -256 recenters for fp16
    precision).
  - VectorE: direction-2 column mins via 2x-mode fp16 elementwise-min
    accumulation; direction-1 row mins via 2x-mode fp16 pair-min of stripe
    halves followed by an in-place binary min-tree.
    NOTE: tensor_tensor_reduce crashes the device at runtime on this silicon
    (3/3 attempts, INTERNAL error) — do not use it here.
  - Host: mean of row mins + mean over cores/partitions of column mins,
    +256 recenter (a2/b2 already folded into the distance tiles).
"""
import numpy as np

N = 16384
D = 128
P = 128
NCORES = 8
MPC = N // NCORES          # rows per core = 2048
MCH = MPC // P             # row chunks per core = 16
NGRP = 8                   # column groups
GW = N // NGRP             # group width = 2048
NPAIR = NGRP // 2          # group pairs per stripe = 4
CENTER = 256.0

_CACHE = {}


def _build(repeat=1):
    from contextlib import ExitStack
    import concourse.bacc as bacc
    import concourse.tile as tile
    from concourse import mybir

    f32 = mybir.dt.float32
    f16 = mybir.dt.float16
    MIN = mybir.AluOpType.min

    nc = bacc.Bacc(trn_type="TRN2", target_bir_lowering=False, debug=False,
                   num_devices=NCORES)

    at_d = nc.dram_tensor("at", [D, MPC], f16, kind="ExternalInput").ap()
    bt_d = nc.dram_tensor("bt", [D, N], f16, kind="ExternalInput").ap()
    a2p_d = nc.dram_tensor("a2p", [D, MPC], f16, kind="ExternalInput").ap()
    ob2p_d = nc.dram_tensor("ob2p", [D, N], f16, kind="ExternalInput").ap()
    rm_d = nc.dram_tensor("rm", [P, MCH], f32, kind="ExternalOutput").ap()
    cm_d = nc.dram_tensor("cm", [P, N], f16, kind="ExternalOutput").ap()

    with tile.TileContext(nc) as tc, ExitStack() as ctx:
        cpool = ctx.enter_context(tc.tile_pool(name="const", bufs=1))
        psum_pool = ctx.enter_context(tc.tile_pool(name="psum", bufs=2, space="PSUM"))
        tpool = ctx.enter_context(tc.tile_pool(name="tg", bufs=8))

        BT = cpool.tile([D, N], f16)
        AT = cpool.tile([D, MPC], f16)
        A2P = cpool.tile([D, MPC], f16)
        OB2P = cpool.tile([D, N], f16)
        ACC = cpool.tile([P, N], f16)
        RM = cpool.tile([P, MCH], f32)
        S = cpool.tile([P, N // 2], f16)

        nc.sync.dma_start(AT[:], at_d[:])
        nc.sync.dma_start(A2P[:], a2p_d[:])
        for g in range(NGRP):
            sl = slice(g * GW, (g + 1) * GW)
            nc.sync.dma_start(BT[:, sl], bt_d[:, sl])
            nc.sync.dma_start(OB2P[:, sl], ob2p_d[:, sl])
        from contextlib import nullcontext
        loop_ctx = tc.For_i(0, repeat, 1) if repeat > 1 else nullcontext()
        with loop_ctx:
            for m in range(MCH):
                msl = slice(m * P, (m + 1) * P)
                for h in range(NPAIR):
                    T = tpool.tile([P, 2 * GW], f16)
                    for half in range(2):
                        g = 2 * h + half
                        ps = psum_pool.tile([P, GW], f32)
                        for k in range(4):
                            nsl = slice(g * GW + k * 512, g * GW + (k + 1) * 512)
                            ksl = slice(k * 512, (k + 1) * 512)
                            nc.tensor.matmul(ps[:, ksl], AT[:, msl], BT[:, nsl],
                                             start=True, stop=False)
                        for k in range(4):
                            nsl = slice(g * GW + k * 512, g * GW + (k + 1) * 512)
                            ksl = slice(k * 512, (k + 1) * 512)
                            nc.tensor.matmul(ps[:, ksl], A2P[:, msl], OB2P[:, nsl],
                                             start=False, stop=True)
                        nc.scalar.activation(T[:, half * GW:(half + 1) * GW], ps[:],
                                             mybir.ActivationFunctionType.Copy,
                                             bias=-CENTER)
                    # direction 2: column-min accumulate over row chunks
                    # (first chunk initializes ACC by copy — no memset needed)
                    gsl2 = slice(2 * h * GW, (2 * h + 2) * GW)
                    if m == 0:
                        nc.vector.tensor_copy(ACC[:, gsl2], T[:])
                    else:
                        nc.vector.tensor_tensor(out=ACC[:, gsl2],
                                                in0=ACC[:, gsl2],
                                                in1=T[:], op=MIN)
                    # direction 1, level 1: pair-min into the stripe buffer
                    hsl = slice(h * GW, (h + 1) * GW)
                    nc.vector.tensor_tensor(out=S[:, hsl], in0=T[:, :GW],
                                            in1=T[:, GW:], op=MIN)
                # direction 1: in-place min-tree over S, then final reduce
                w = N // 4
                while w >= 256:
                    nc.vector.tensor_tensor(out=S[:, :w], in0=S[:, :w],
                                            in1=S[:, w:2 * w], op=MIN)
                    w //= 2
                nc.vector.tensor_reduce(out=RM[:, m:m + 1], in_=S[:, :256],
                                        axis=mybir.AxisListType.X, op=MIN)

        nc.sync.dma_start(rm_d[:], RM[:])
        for g in range(NGRP):
            sl = slice(g * GW, (g + 1) * GW)
            nc.sync.dma_start(cm_d[:, sl], ACC[:, sl])

    nc.compile()
    return nc


def kernel(point_cloud1: np.ndarray, point_cloud2: np.ndarray) -> np.ndarray:
    import os
    from concourse.bass_utils import run_bass_kernel_spmd

    if "nc" not in _CACHE:
        _CACHE["nc"] = _build()
    nc = _CACHE["nc"]

    pc1 = np.ascontiguousarray(np.asarray(point_cloud1).reshape(N, D),
                               dtype=np.float32)
    pc2 = np.ascontiguousarray(np.asarray(point_cloud2).reshape(N, D),
                               dtype=np.float32)
    a2 = (pc1.astype(np.float64) ** 2).sum(1).astype(np.float32)
    b2 = (pc2.astype(np.float64) ** 2).sum(1).astype(np.float32)

    bt = np.ascontiguousarray(pc2.T).astype(np.float16)
    ob2p = np.zeros((D, N), np.float16)
    ob2p[0] = 1.0
    ob2p[1] = b2.astype(np.float16)

    in_maps = []
    for c in range(NCORES):
        rs = slice(c * MPC, (c + 1) * MPC)
        a2p = np.zeros((D, MPC), np.float16)
        a2p[0] = a2[rs].astype(np.float16)
        a2p[1] = 1.0
        in_maps.append({
            "at": np.ascontiguousarray(-2.0 * pc1[rs].T).astype(np.float16),
            "bt": bt,
            "a2p": a2p,
            "ob2p": ob2p,
        })

    trace = os.environ.get("KERNEL_TRACE", "0") == "1"
    if trace:
        try:
            import antenv.axon_hooks  # noqa: F401
        except ImportError:
            trace = False
    res = run_bass_kernel_spmd(nc, in_maps, core_ids=list(range(NCORES)),
                               trace=trace)
    _CACHE["last_exec_ns"] = res.exec_time_ns

    rowmins = []
    colmins = []
    for r in res.results:
        # rm[p, m] is the row-min of core row m*128+p, minus CENTER
        rowmins.append(r["rm"].T.reshape(MPC))
        colmins.append(r["cm"].astype(np.float32))
    min1 = np.concatenate(rowmins) + CENTER
    min2 = np.concatenate(colmins, axis=0).min(axis=0) + CENTER
    out = np.float64(min1.mean()) + np.float64(min2.mean())
    return np.asarray(out, dtype=np.float32)
